# revision 1
# baseline (speedup 1.0000x reference)
"""AttentionHead kernel for Trainium2, 8 NeuronCores, data-parallel over batch.

Problem (fixed shapes):
    input_tensor [8, 2048, 1024] f32, attention_mask [8, 2048] int64 (0/1),
    Wq/Wk/Wv [1024, 128] f32, bq/bk/bv [128] f32.
    out = softmax(mask(Q @ K^T / sqrt(2048))) @ V    -> [8, 2048, 128] f32

Sharding: one batch element per core (B == n_cores == 8). No collectives.

Per-core device kernel (bf16 inputs, f32 accumulation):
  - Host pre-transposes X -> XT [1024, 2048] (8 per-chunk DRAM->SBUF tiles so
    the PE can start as soon as chunk 0 lands) and folds 1/sqrt(S) into Wq/bq.
  - QT/KT/VT [128(e), 2048(tok)] = W^T @ XT (PE, K=1024, N=512 matmuls).
  - V [2048(key), 128] from VT via 16 PE transposes.
  - Per query block t (512 queries):
      S^T tiles [128(key), 512(q)] (PE, N=512); exp on ScalarE over 2-bank
      PSUM groups (no max-subtraction: |scores| <= ~2 by construction);
      mask folded into E by per-partition multiply with mask(key) in {0,1};
      numerator OT [128(e), 512(q)] = sum_j V_j^T @ E_j (PE, N=512, V_j
      stationary); denominator = ones^T @ (DVE pairwise tree-sum of E_j)
      (one [K=128,M=1,N=512] matmul); reciprocal on DVE, gpsimd
      partition_broadcast, final DVE multiply. The denominator/normalize
      chain is deferred one query block so the PE never waits on ScalarE/DVE.
  - Output written as OT [128, 2048]; host transposes to [2048, 128].
"""

import sys
import types

for _p in ("/opt/trn_rl_repo", "/root/.axon_site/_ro/trn_rl_repo"):
    if _p not in sys.path:
        sys.path.append(_p)

import numpy as np
import ml_dtypes

B, S, DIN, DOUT = 8, 2048, 1024, 128
NCHUNK = DIN // 128          # 8 contraction chunks
NKEY = S // 128              # 16 key chunks
QBLK = 512                   # query block (free dim of S^T / OT matmuls)
NQB = S // QBLK              # 4 query blocks
STG = 2                      # key chunks per exp group ([128, STG*512] psum)
NGRP = NKEY // STG           # 8 exp groups per query block

BF16 = ml_dtypes.bfloat16


def _build():
    import concourse.bass as bass
    import concourse.tile as tile
    from concourse import bacc, mybir
    from concourse.masks import make_identity

    f32 = mybir.dt.float32
    bf16 = mybir.dt.bfloat16
    Exp = mybir.ActivationFunctionType.Exp

    nc = bacc.Bacc("TRN2", target_bir_lowering=False, debug=False, num_devices=B)

    xt_d = nc.dram_tensor("xt", [DIN, S], bf16, kind="ExternalInput")
    wq_d = nc.dram_tensor("wq", [DIN, DOUT], bf16, kind="ExternalInput")
    wk_d = nc.dram_tensor("wk", [DIN, DOUT], bf16, kind="ExternalInput")
    wv_d = nc.dram_tensor("wv", [DIN, DOUT], bf16, kind="ExternalInput")
    bq_d = nc.dram_tensor("bq", [1, DOUT], bf16, kind="ExternalInput")
    bk_d = nc.dram_tensor("bk", [1, DOUT], bf16, kind="ExternalInput")
    bv_d = nc.dram_tensor("bv", [1, DOUT], bf16, kind="ExternalInput")
    m01_d = nc.dram_tensor("m01", [128, NKEY], f32, kind="ExternalInput")
    out_d = nc.dram_tensor("out", [DOUT, S], f32, kind="ExternalOutput")

    with tile.TileContext(nc) as tc:
        with (
            tc.tile_pool(name="persist", bufs=1) as pp,
            tc.tile_pool(name="epool", bufs=2 * NGRP) as ep,
            tc.tile_pool(name="tree", bufs=2) as tp,
            tc.tile_pool(name="normp", bufs=2) as rp,
            tc.tile_pool(name="outp", bufs=2) as op,
        ):
            xts = [pp.tile([128, S], bf16, tag=f"xt{c}", name=f"xt{c}")
                   for c in range(NCHUNK)]
            wq = pp.tile([128, NCHUNK * DOUT], bf16, tag="wq")
            wk = pp.tile([128, NCHUNK * DOUT], bf16, tag="wk")
            wv = pp.tile([128, NCHUNK * DOUT], bf16, tag="wv")
            bq = pp.tile([1, DOUT], bf16, tag="bq")
            bk = pp.tile([1, DOUT], bf16, tag="bk")
            bv = pp.tile([1, DOUT], bf16, tag="bv")
            m01 = pp.tile([128, NKEY], f32, tag="m01")
            ones = pp.tile([1, QBLK], bf16, tag="ones")
            ocol = pp.tile([128, 1], bf16, tag="ocol")
            ident = pp.tile([128, 128], bf16, tag="ident")
            qt = pp.tile([128, S], bf16, tag="qt")
            kt = pp.tile([128, S], bf16, tag="kt")
            vt = pp.tile([128, S], bf16, tag="vt")
            vn = pp.tile([128, NKEY * 128], bf16, tag="vn")

            nc.sync.dma_start(wq[:].rearrange("p (c e) -> p c e", c=NCHUNK),
                              wq_d.ap().rearrange("(c p) e -> p c e", p=128))
            nc.sync.dma_start(wk[:].rearrange("p (c e) -> p c e", c=NCHUNK),
                              wk_d.ap().rearrange("(c p) e -> p c e", p=128))
            nc.sync.dma_start(wv[:].rearrange("p (c e) -> p c e", c=NCHUNK),
                              wv_d.ap().rearrange("(c p) e -> p c e", p=128))
            nc.sync.dma_start(bq[:], bq_d.ap())
            nc.sync.dma_start(bk[:], bk_d.ap())
            nc.sync.dma_start(bv[:], bv_d.ap())
            nc.sync.dma_start(m01[:], m01_d.ap())
            nc.vector.memset(ones[:], 1.0)
            nc.vector.memset(ocol[:], 1.0)
            make_identity(nc, ident[:])

            xt3 = xt_d.ap().rearrange("(c p) m -> p c m", p=128)
            for c in range(NCHUNK):
                nc.sync.dma_start(xts[c][:], xt3[:, c, :])

            # ---- Phase A: QT / KT / VT projections ----
            with tc.tile_pool(name="ps_a", bufs=NQB, space="PSUM") as ps_a:
                for w, bias, dst, nm in ((wq, bq, qt, "q"), (wk, bk, kt, "k"),
                                         (wv, bv, vt, "v")):
                    ps = [ps_a.tile([128, QBLK], f32, tag="a", name=f"pa{nm}{t}")
                          for t in range(NQB)]
                    for c in range(NCHUNK):
                        for t in range(NQB):
                            nc.tensor.matmul(
                                ps[t][:],
                                w[:, c * DOUT:(c + 1) * DOUT],
                                xts[c][:, t * QBLK:(t + 1) * QBLK],
                                start=(c == 0), stop=False,
                            )
                    for t in range(NQB):
                        nc.tensor.matmul(ps[t][:], bias[:], ones[:],
                                         start=False, stop=True)
                    for t in range(NQB):
                        nc.vector.tensor_copy(dst[:, t * QBLK:(t + 1) * QBLK],
                                              ps[t][:])

            # ---- Phase B: V natural layout via PE transpose ----
            with tc.tile_pool(name="ps_tr", bufs=3, space="PSUM") as ps_tr:
                for k in range(NKEY):
                    ptr = ps_tr.tile([128, 128], bf16, tag="tr")
                    nc.tensor.transpose(ptr[:], vt[:, k * 128:(k + 1) * 128],
                                        ident[:])
                    nc.vector.tensor_copy(vn[:, k * 128:(k + 1) * 128], ptr[:])

            # ---- Phase C: attention ----
            with (
                tc.tile_pool(name="ps_st", bufs=2, space="PSUM") as ps_st,
                tc.tile_pool(name="ps_o", bufs=2, space="PSUM") as ps_o,
                tc.tile_pool(name="ps_m", bufs=1, space="PSUM") as ps_m,
            ):
                def finish(st):
                    t, pot, pd = st
                    rd = rp.tile([1, QBLK], f32, tag="rd", name=f"rd{t}")
                    nc.vector.reciprocal(rd[:], pd[:])
                    rdb = rp.tile([128, QBLK], f32, tag="rdb", name=f"rdb{t}")
                    nc.gpsimd.partition_broadcast(rdb[:], rd[:])
                    osb = op.tile([128, QBLK], f32, tag="osb", name=f"osb{t}")
                    nc.vector.tensor_mul(osb[:], pot[:], rdb[:])
                    nc.sync.dma_start(out_d.ap()[:, t * QBLK:(t + 1) * QBLK],
                                      osb[:])

                pending = None
                for t in range(NQB):
                    egs = []
                    for g in range(NGRP):
                        pst = ps_st.tile([128, STG * QBLK], f32, tag="st")
                        for jj in range(STG):
                            j = g * STG + jj
                            nc.tensor.matmul(
                                pst[:, jj * QBLK:(jj + 1) * QBLK],
                                kt[:, j * 128:(j + 1) * 128],
                                qt[:, t * QBLK:(t + 1) * QBLK],
                                start=True, stop=True,
                            )
                        eg = ep.tile([128, STG * QBLK], bf16, tag="e",
                                     name=f"eg{t}_{g}")
                        nc.scalar.activation(eg[:], pst[:], Exp)
                        for jj in range(STG):
                            j = g * STG + jj
                            sl = eg[:, jj * QBLK:(jj + 1) * QBLK]
                            nc.vector.tensor_scalar_mul(sl, sl, m01[:, j:j + 1])
                        egs.append(eg)

                    # numerator: OT += V_j^T @ E_j  (V_j stationary, N=512)
                    pot = ps_o.tile([128, QBLK], f32, tag="o", name=f"pot{t}")
                    for j in range(NKEY):
                        g, jj = j // STG, j % STG
                        nc.tensor.matmul(
                            pot[:],
                            vn[:, j * 128:(j + 1) * 128],
                            egs[g][:, jj * QBLK:(jj + 1) * QBLK],
                            start=(j == 0), stop=(j == NKEY - 1),
                        )

                    # denominator: pairwise DVE tree over the 16 E slices,
                    # then ones^T @ esum on PE.
                    lvl = []
                    for g in range(NGRP):
                        a = tp.tile([128, QBLK], bf16, tag=f"t1_{g % 4}",
                                    name=f"a{t}_{g}", bufs=3)
                        nc.vector.tensor_add(a[:], egs[g][:, :QBLK],
                                             egs[g][:, QBLK:])
                        lvl.append(a)
                    while len(lvl) > 1:
                        nxt = []
                        for i in range(0, len(lvl), 2):
                            a = tp.tile([128, QBLK], bf16,
                                        tag=f"t2_{len(lvl)}_{i % 2}",
                                        name=f"s{t}_{len(lvl)}_{i}", bufs=2)
                            nc.vector.tensor_add(a[:], lvl[i][:], lvl[i + 1][:])
                            nxt.append(a)
                        lvl = nxt
                    pd = ps_m.tile([1, QBLK], f32, tag="d", name=f"pd{t}")
                    nc.tensor.matmul(pd[:], ocol[:], lvl[0][:],
                                     start=True, stop=True)

                    if pending is not None:
                        finish(pending)
                    pending = (t, pot, pd)
                finish(pending)

    nc.compile()
    return nc


_NC = None


def _get_nc():
    global _NC
    if _NC is None:
        _NC = _build()
    return _NC


def _prep_in_maps(input_tensor, attention_mask, Wq, bq, Wk, bk, Wv, bv):
    scale = np.float32(1.0 / np.sqrt(np.float32(S)))
    wq_h = (np.asarray(Wq, np.float32) * scale).astype(BF16)
    wk_h = np.asarray(Wk, np.float32).astype(BF16)
    wv_h = np.asarray(Wv, np.float32).astype(BF16)
    bq_h = (np.asarray(bq, np.float32) * scale).astype(BF16).reshape(1, DOUT)
    bk_h = np.asarray(bk, np.float32).astype(BF16).reshape(1, DOUT)
    bv_h = np.asarray(bv, np.float32).astype(BF16).reshape(1, DOUT)

    x = np.asarray(input_tensor, np.float32)
    m = np.asarray(attention_mask)
    in_maps = []
    for b in range(B):
        xt_h = np.ascontiguousarray(x[b].T).astype(BF16)            # [DIN, S]
        m01_h = np.ascontiguousarray(
            m[b].astype(np.float32).reshape(NKEY, 128).T)           # [128, NKEY]
        in_maps.append({
            "xt": xt_h, "wq": wq_h, "wk": wk_h, "wv": wv_h,
            "bq": bq_h, "bk": bk_h, "bv": bv_h, "m01": m01_h,
        })
    return in_maps


def run(in_maps, trace=False, **kwargs):
    from concourse.bass_utils import run_bass_kernel_spmd

    nc = _get_nc()
    return run_bass_kernel_spmd(
        nc, in_maps, core_ids=list(range(B)), trace=trace, **kwargs
    )


def kernel(input_tensor, attention_mask, Wq, bq, Wk, bk, Wv, bv):
    in_maps = _prep_in_maps(
        input_tensor, attention_mask, Wq, bq, Wk, bk, Wv, bv)
    res = run(in_maps, trace=False)
    out = np.stack([res.results[b]["out"].T for b in range(B)])
    return np.ascontiguousarray(out.astype(np.float32))



# revision 2
# speedup vs baseline: 1.1563x; 1.1563x over previous
"""AttentionHead kernel for Trainium2, 8 NeuronCores, data-parallel over batch.

Problem (fixed shapes):
    input_tensor [8, 2048, 1024] f32, attention_mask [8, 2048] int64 (0/1),
    Wq/Wk/Wv [1024, 128] f32, bq/bk/bv [128] f32.
    out = softmax(mask(Q @ K^T / sqrt(2048))) @ V    -> [8, 2048, 128] f32

Sharding: one batch element per core (B == n_cores == 8). No collectives.

Per-core device kernel (bf16 inputs, f32 accumulation), v2:
  - Host pre-transposes X -> XT [1024, 2048] bf16, prepacks W into the SBUF
    layout [128, 8*128] (contiguous 2KB partition lines -> fast DMA), folds
    1/sqrt(S) into Wq/bq.
  - DMA issue split across the two HWDGE queues (sync: wk/wq + X halves in
    arrival order; scalar: wv/bcol/m01) so the PE starts at ~2us. A dummy
    exp at t=0 preloads the ACT exp table.
  - Pass 1 (DMA-paced): KT (all 4 blocks) + QT block 0, per-chunk.
    Bias is folded into the PSUM->SBUF cast (tensor_scalar add, per-partition)
    instead of K=1 matmuls.
  - Scores S^T [128(key), 512(q)] = KT_j^T-slice @ QT-block (PE, N=512),
    written per pair of key chunks into a [128, 1024] PSUM tile; exp on
    ScalarE (no max-subtraction: |scores| <= ~1.5 by construction), UNMASKED.
  - Mask handling: V rows for masked keys are zeroed (fused into the
    V-transpose PSUM->SBUF copy as a per-partition multiply), and the softmax
    denominator is a masked running sum over E (fused multiply-add
    scalar_tensor_tensor on DVE), so the exp needs no mask at all.
  - Score groups for query blocks 0-1 are interleaved into the remaining
    projection matmuls (QT blocks 1-3, VT, V transposes) so ScalarE's exp
    stream starts ~13us in and stays saturated; AV matmuls for block t are
    interleaved behind score matmuls of block t+2.
  - Denominator: ones^T @ (masked running sum) (one [K=128,M=1,N=512]
    matmul), reciprocal_approx_fast on DVE, gpsimd partition_broadcast,
    final DVE multiply, DMA out. The chain runs concurrently with later
    AV matmuls.
  - Output written as OT [128, 2048]; host transposes to [2048, 128].
"""

import sys

for _p in ("/opt/trn_rl_repo", "/root/.axon_site/_ro/trn_rl_repo"):
    if _p not in sys.path:
        sys.path.append(_p)

import numpy as np
import ml_dtypes

B, S, DIN, DOUT = 8, 2048, 1024, 128
NCHUNK = DIN // 128          # 8 contraction chunks
NKEY = S // 128              # 16 key chunks
QBLK = 512                   # query block (free dim of S^T / OT matmuls)
NQB = S // QBLK              # 4 query blocks
NGRP = NKEY // 2             # 8 exp groups (2 key chunks each) per query block

BF16 = ml_dtypes.bfloat16


def _build():
    import concourse.bass as bass
    import concourse.tile as tile
    from concourse import bacc, mybir
    from concourse.masks import make_identity

    f32 = mybir.dt.float32
    bf16 = mybir.dt.bfloat16
    Exp = mybir.ActivationFunctionType.Exp
    Alu = mybir.AluOpType

    nc = bacc.Bacc("TRN2", target_bir_lowering=False, debug=False, num_devices=B)

    xt_d = nc.dram_tensor("xt", [DIN, S], bf16, kind="ExternalInput")
    wq_d = nc.dram_tensor("wq", [128, DIN], bf16, kind="ExternalInput")
    wk_d = nc.dram_tensor("wk", [128, DIN], bf16, kind="ExternalInput")
    wv_d = nc.dram_tensor("wv", [128, DIN], bf16, kind="ExternalInput")
    bcol_d = nc.dram_tensor("bcol", [128, 4], f32, kind="ExternalInput")
    m01_d = nc.dram_tensor("m01", [128, NKEY], f32, kind="ExternalInput")
    out_d = nc.dram_tensor("out", [DOUT, S], f32, kind="ExternalOutput")

    with tile.TileContext(nc) as tc:
        with (
            tc.tile_pool(name="persist", bufs=1) as pp,
            tc.tile_pool(name="epool", bufs=16) as ep,
            tc.tile_pool(name="tree", bufs=3) as tp,
            tc.tile_pool(name="normp", bufs=2) as rp,
            tc.tile_pool(name="outp", bufs=2) as op,
        ):
            xts = [pp.tile([128, S], bf16, tag=f"xt{c}", name=f"xt{c}")
                   for c in range(NCHUNK)]
            wq = pp.tile([128, DIN], bf16, tag="wq")
            wk = pp.tile([128, DIN], bf16, tag="wk")
            wv = pp.tile([128, DIN], bf16, tag="wv")
            bcol = pp.tile([128, 4], f32, tag="bcol")
            m01 = pp.tile([128, NKEY], f32, tag="m01")
            ocol = pp.tile([128, 1], bf16, tag="ocol")
            ident = pp.tile([128, 128], bf16, tag="ident")
            qt = pp.tile([128, S], bf16, tag="qt")
            kt = pp.tile([128, S], bf16, tag="kt")
            vt = pp.tile([128, S], bf16, tag="vt")
            vn = pp.tile([128, S], bf16, tag="vn")
            wrm_i = pp.tile([1, 32], f32, tag="wrm_i")
            wrm_o = pp.tile([1, 32], f32, tag="wrm_o")

            # exp table preload (overlaps the input DMA)
            nc.vector.memset(wrm_i[:], 0.0)
            nc.scalar.activation(wrm_o[:], wrm_i[:], Exp)

            nc.vector.memset(ocol[:], 1.0)
            make_identity(nc, ident[:])

            # DMAs. sync queue: wk, wq, X chunk halves in consumption order.
            # scalar queue: wv, bcol, m01 (needed later; issue in parallel).
            xt3 = xt_d.ap().rearrange("(c p) m -> p c m", p=128)
            nc.sync.dma_start(wk[:], wk_d.ap())
            nc.sync.dma_start(xts[0][:, 0:1024], xt3[:, 0, 0:1024])
            nc.sync.dma_start(wq[:], wq_d.ap())
            nc.sync.dma_start(xts[0][:, 1024:2048], xt3[:, 0, 1024:2048])
            for c in range(1, NCHUNK):
                nc.sync.dma_start(xts[c][:, 0:1024], xt3[:, c, 0:1024])
                nc.sync.dma_start(xts[c][:, 1024:2048], xt3[:, c, 1024:2048])
            nc.scalar.dma_start(wv[:], wv_d.ap())
            nc.scalar.dma_start(bcol[:], bcol_d.ap())
            nc.scalar.dma_start(m01[:], m01_d.ap())

            def cast_bias(dst_sl, src, col):
                nc.vector.tensor_scalar_add(dst_sl, src, bcol[:, col:col + 1])

            egs = {}      # (t, g) -> E tile [128, 1024] bf16
            rlast = {}    # t -> latest running-sum tile

            # ---------- pass 1: KT (all blocks) + QT block 0, DMA-paced ----
            with tc.tile_pool(name="psA1", bufs=1, space="PSUM") as psA1:
                psK = [psA1.tile([128, QBLK], f32, tag=f"pk{t}", name=f"pk{t}")
                       for t in range(NQB)]
                psQ0 = psA1.tile([128, QBLK], f32, tag="pq0")
                for c in range(NCHUNK):
                    wks = wk[:, c * 128:(c + 1) * 128]
                    wqs = wq[:, c * 128:(c + 1) * 128]
                    st, sp = (c == 0), (c == NCHUNK - 1)
                    nc.tensor.matmul(psK[0][:], wks, xts[c][:, 0:512],
                                     start=st, stop=sp)
                    nc.tensor.matmul(psK[1][:], wks, xts[c][:, 512:1024],
                                     start=st, stop=sp)
                    nc.tensor.matmul(psQ0[:], wqs, xts[c][:, 0:512],
                                     start=st, stop=sp)
                    nc.tensor.matmul(psK[2][:], wks, xts[c][:, 1024:1536],
                                     start=st, stop=sp)
                    nc.tensor.matmul(psK[3][:], wks, xts[c][:, 1536:2048],
                                     start=st, stop=sp)
                for t in range(NQB):
                    cast_bias(kt[:, t * QBLK:(t + 1) * QBLK], psK[t][:], 1)
                cast_bias(qt[:, 0:QBLK], psQ0[:], 0)

            # ---------- phase C pools (ps_st lives through the R region) ----
            with tc.tile_pool(name="ps_st", bufs=2, space="PSUM") as ps_st:

                def emit_score_group(t, g):
                    pst = ps_st.tile([128, 2 * QBLK], f32, tag="st",
                                     name=f"pst{t}_{g}")
                    for jj in (0, 1):
                        j = 2 * g + jj
                        nc.tensor.matmul(
                            pst[:, jj * QBLK:(jj + 1) * QBLK],
                            kt[:, j * 128:(j + 1) * 128],
                            qt[:, t * QBLK:(t + 1) * QBLK],
                            start=True, stop=True,
                        )
                    eg = ep.tile([128, 2 * QBLK], bf16, tag="e",
                                 name=f"eg{t}_{g}")
                    nc.scalar.activation(eg[:], pst[:], Exp)
                    egs[(t, g)] = eg

                def emit_stt(t, j):
                    eg = egs[(t, j // 2)]
                    half = eg[:, (j % 2) * QBLK:((j % 2) + 1) * QBLK]
                    r = tp.tile([128, QBLK], bf16, tag="r", name=f"r{t}_{j}")
                    if j == 0:
                        nc.vector.tensor_scalar_mul(r[:], half, m01[:, 0:1])
                    else:
                        nc.vector.scalar_tensor_tensor(
                            r[:], half, m01[:, j:j + 1], rlast[t][:],
                            op0=Alu.mult, op1=Alu.add)
                    rlast[t] = r

                # ---- R region: rest of projections + V transposes,
                # interleaved with score groups for blocks 0 and 1 ----
                with (
                    tc.tile_pool(name="psA2", bufs=2, space="PSUM") as psA2,
                    tc.tile_pool(name="ps_tr", bufs=2, space="PSUM") as ps_tr,
                ):
                    def proj8(w, t, dst, col, s_before):
                        """8 proj MMs for (w, block t), with score groups
                        interleaved before each half."""
                        pr = psA2.tile([128, QBLK], f32, tag="pr",
                                       name=f"pr_{dst!s}_{t}")
                        for half in range(2):
                            if s_before:
                                emit_score_group(*s_before.pop(0))
                            for c in range(4 * half, 4 * half + 4):
                                nc.tensor.matmul(
                                    pr[:],
                                    w[:, c * 128:(c + 1) * 128],
                                    xts[c][:, t * QBLK:(t + 1) * QBLK],
                                    start=(c == 0), stop=(c == NCHUNK - 1),
                                )
                        cast_bias(dst[:, t * QBLK:(t + 1) * QBLK], pr[:], col)

                    def tr4(t):
                        """transpose V key chunks 4t..4t+3 (masked copy)."""
                        for k in range(4 * t, 4 * t + 4):
                            ptr = ps_tr.tile([128, 128], bf16, tag="tr",
                                             name=f"tr{k}")
                            nc.tensor.transpose(
                                ptr[:], vt[:, k * 128:(k + 1) * 128], ident[:])
                            nc.vector.tensor_scalar_mul(
                                vn[:, k * 128:(k + 1) * 128], ptr[:],
                                m01[:, k:k + 1])

                    sq = [(0, g) for g in range(NGRP)] + \
                         [(1, g) for g in range(NGRP)]
                    proj8(wq, 1, qt, 0, sq)   # consumes s(0,0), s(0,1)
                    proj8(wq, 2, qt, 0, sq)   # s(0,2), s(0,3)
                    proj8(wq, 3, qt, 0, sq)   # s(0,4), s(0,5)
                    proj8(wv, 0, vt, 2, sq)   # s(0,6), s(0,7)
                    for j in range(0, 2):
                        emit_stt(0, j)
                    tr4(0)
                    proj8(wv, 1, vt, 2, sq)   # s(1,0), s(1,1)
                    for j in range(2, 6):
                        emit_stt(0, j)
                    tr4(1)
                    proj8(wv, 2, vt, 2, sq)   # s(1,2), s(1,3)
                    for j in range(6, 10):
                        emit_stt(0, j)
                    tr4(2)
                    proj8(wv, 3, vt, 2, sq)   # s(1,4), s(1,5)
                    for j in range(10, 14):
                        emit_stt(0, j)
                    tr4(3)
                    while sq:
                        emit_score_group(*sq.pop(0))   # s(1,6), s(1,7)
                    for j in range(14, 16):
                        emit_stt(0, j)
                    for j in range(0, 4):
                        emit_stt(1, j)

                # ---- steady phase C: scores t+2 interleaved with AV t ----
                with (
                    tc.tile_pool(name="ps_o", bufs=2, space="PSUM") as ps_o,
                    tc.tile_pool(name="ps_m", bufs=2, space="PSUM") as ps_m,
                ):
                    pots = {}

                    def emit_av(t, j):
                        if j == 0:
                            pots[t] = ps_o.tile([128, QBLK], f32, tag="o",
                                                name=f"pot{t}")
                        eg = egs[(t, j // 2)]
                        nc.tensor.matmul(
                            pots[t][:],
                            vn[:, j * 128:(j + 1) * 128],
                            eg[:, (j % 2) * QBLK:((j % 2) + 1) * QBLK],
                            start=(j == 0), stop=(j == NKEY - 1),
                        )

                    def emit_pd_finish(t):
                        pd = ps_m.tile([1, QBLK], f32, tag="d", name=f"pd{t}")
                        nc.tensor.matmul(pd[:], ocol[:], rlast[t][:],
                                         start=True, stop=True)
                        rdc = rp.tile([1, QBLK], f32, tag="rdc",
                                      name=f"rdc{t}")
                        nc.vector.reciprocal_approx_fast(rdc[:], pd[:])
                        rdb = rp.tile([128, QBLK], f32, tag="rdb",
                                      name=f"rdb{t}")
                        nc.gpsimd.partition_broadcast(rdb[:], rdc[:])
                        osb = op.tile([128, QBLK], f32, tag="osb",
                                      name=f"osb{t}")
                        nc.vector.tensor_mul(osb[:], pots[t][:], rdb[:])
                        nc.sync.dma_start(
                            out_d.ap()[:, t * QBLK:(t + 1) * QBLK], osb[:])

                    # block 0 AV behind block 2 scores
                    for g in range(NGRP):
                        emit_score_group(2, g)
                        emit_av(0, 2 * g)
                        emit_av(0, 2 * g + 1)
                    for j in range(4, 16):
                        emit_stt(1, j)
                    emit_pd_finish(0)
                    # block 1 AV behind block 3 scores
                    for g in range(NGRP):
                        emit_score_group(3, g)
                        emit_av(1, 2 * g)
                        emit_av(1, 2 * g + 1)
                    for j in range(0, 16):
                        emit_stt(2, j)
                    emit_pd_finish(1)
                    # trailing AV for blocks 2 and 3
                    for j in range(NKEY):
                        emit_av(2, j)
                    for j in range(0, 16):
                        emit_stt(3, j)
                    emit_pd_finish(2)
                    for j in range(NKEY):
                        emit_av(3, j)
                    emit_pd_finish(3)

    nc.compile()
    return nc


_NC = None


def _get_nc():
    global _NC
    if _NC is None:
        _NC = _build()
    return _NC


def _prep_in_maps(input_tensor, attention_mask, Wq, bq, Wk, bk, Wv, bv):
    scale = np.float32(1.0 / np.sqrt(np.float32(S)))

    def pack_w(w, sc=None):
        w = np.asarray(w, np.float32)
        if sc is not None:
            w = w * sc
        # [1024, 128] -> [128, 8*128]: row c*128+p, col e -> [p, c*128+e]
        return np.ascontiguousarray(
            w.reshape(NCHUNK, 128, DOUT).transpose(1, 0, 2).reshape(128, DIN)
        ).astype(BF16)

    wq_h = pack_w(Wq, scale)
    wk_h = pack_w(Wk)
    wv_h = pack_w(Wv)
    bcol_h = np.zeros((128, 4), np.float32)
    bcol_h[:, 0] = np.asarray(bq, np.float32) * scale
    bcol_h[:, 1] = np.asarray(bk, np.float32)
    bcol_h[:, 2] = np.asarray(bv, np.float32)

    x = np.asarray(input_tensor, np.float32)
    m = np.asarray(attention_mask)
    in_maps = []
    for b in range(B):
        xt_h = np.ascontiguousarray(x[b].T).astype(BF16)            # [DIN, S]
        m01_h = np.ascontiguousarray(
            m[b].astype(np.float32).reshape(NKEY, 128).T)           # [128, NKEY]
        in_maps.append({
            "xt": xt_h, "wq": wq_h, "wk": wk_h, "wv": wv_h,
            "bcol": bcol_h, "m01": m01_h,
        })
    return in_maps


def run(in_maps, trace=False, **kwargs):
    from concourse.bass_utils import run_bass_kernel_spmd

    nc = _get_nc()
    return run_bass_kernel_spmd(
        nc, in_maps, core_ids=list(range(B)), trace=trace, **kwargs
    )


def kernel(input_tensor, attention_mask, Wq, bq, Wk, bk, Wv, bv):
    in_maps = _prep_in_maps(
        input_tensor, attention_mask, Wq, bq, Wk, bk, Wv, bv)
    res = run(in_maps, trace=False)
    out = np.stack([res.results[b]["out"].T for b in range(B)])
    return np.ascontiguousarray(out.astype(np.float32))


# revision 5
# speedup vs baseline: 1.4014x; 1.2119x over previous
"""AttentionHead kernel for Trainium2, 8 NeuronCores, data-parallel over batch.

Problem (fixed shapes):
    input_tensor [8, 2048, 1024] f32, attention_mask [8, 2048] int64 (0/1),
    Wq/Wk/Wv [1024, 128] f32, bq/bk/bv [128] f32.
    out = softmax(mask(Q @ K^T / sqrt(2048))) @ V    -> [8, 2048, 128] f32

Sharding: one batch element per core (B == n_cores == 8). No collectives.

Per-core device kernel (bf16 inputs, f32 accumulation), v3:
  - Host pre-transposes X -> XT [1024, 2048] bf16, prepacks W into the SBUF
    layout [128, 8*128] (contiguous 2KB partition lines -> fast DMA), folds
    1/sqrt(S) into Wq/bq.
  - DMA issue is split across the two HWDGE queues (sync + scalar) so issue
    overhead (~0.6us per DMA instruction) overlaps; a dummy exp at t=0
    preloads the ACT exp table during the X DMA.
  - Pass 1 (DMA-paced, per chunk): KT all 4 blocks + QT blocks 0,1.
    Bias is folded into the PSUM->SBUF cast (per-partition tensor_scalar).
  - Score/exp tiles cover ONE key chunk x TWO query blocks [128, 1024]:
    within a tile the mask depends only on the partition (the key), so the
    attention mask folds into the exp's per-partition bias (0 or -50) and
    E comes out of ScalarE already masked. 32 activations total; no
    max-subtraction (|scores| <= ~1.5 by construction).
  - Softmax denominator: plain running TENSOR_TENSOR adds over the 16
    pre-masked E tiles per block pair, one [K=128,M=1,N=512] matmul with a
    ones column, reciprocal_approx_fast (DVE), gpsimd partition_broadcast,
    final DVE multiply. The chain is emitted early so it overlaps AV.
  - Emission interleaves: scores(blocks 0,1) into the remaining projections;
    scores(blocks 2,3) + AV(0..3) in one loop, AV of blocks 2,3 lagging two
    key chunks so every engine stays busy. PE never idles > ~1us, so the
    HAM clock gate stays open.
  - Output written as OT [128, 2048]; host transposes to [2048, 128].
"""

import sys

for _p in ("/opt/trn_rl_repo", "/root/.axon_site/_ro/trn_rl_repo"):
    if _p not in sys.path:
        sys.path.append(_p)

import numpy as np
import ml_dtypes

B, S, DIN, DOUT = 8, 2048, 1024, 128
NCHUNK = DIN // 128          # 8 contraction chunks
NKEY = S // 128              # 16 key chunks
QBLK = 512                   # query block (free dim of S^T / OT matmuls)
NQB = S // QBLK              # 4 query blocks

BF16 = ml_dtypes.bfloat16


def _build():
    import concourse.bass as bass
    import concourse.tile as tile
    from concourse import bacc, mybir
    from concourse.masks import make_identity

    f32 = mybir.dt.float32
    bf16 = mybir.dt.bfloat16
    Exp = mybir.ActivationFunctionType.Exp

    nc = bacc.Bacc("TRN2", target_bir_lowering=False, debug=False, num_devices=B)

    xt_d = nc.dram_tensor("xt", [DIN, S], bf16, kind="ExternalInput")
    wq_d = nc.dram_tensor("wq", [128, DIN], bf16, kind="ExternalInput")
    wk_d = nc.dram_tensor("wk", [128, DIN], bf16, kind="ExternalInput")
    wv_d = nc.dram_tensor("wv", [128, DIN], bf16, kind="ExternalInput")
    bcol_d = nc.dram_tensor("bcol", [128, 4], f32, kind="ExternalInput")
    mb_d = nc.dram_tensor("mb", [128, NKEY], f32, kind="ExternalInput")
    out_d = nc.dram_tensor("out", [DOUT, S], f32, kind="ExternalOutput")

    with tile.TileContext(nc) as tc:
        with (
            tc.tile_pool(name="persist", bufs=1) as pp,
            tc.tile_pool(name="epool", bufs=20) as ep,
            tc.tile_pool(name="tree", bufs=6) as tp,
            tc.tile_pool(name="normp", bufs=2) as rp,
            tc.tile_pool(name="outp", bufs=2) as op,
        ):
            xts = [pp.tile([128, S], bf16, tag=f"xt{c}", name=f"xt{c}")
                   for c in range(NCHUNK)]
            wq = pp.tile([128, DIN], bf16, tag="wq")
            wk = pp.tile([128, DIN], bf16, tag="wk")
            wv = pp.tile([128, DIN], bf16, tag="wv")
            bcol = pp.tile([128, 4], f32, tag="bcol")
            mb = pp.tile([128, NKEY], f32, tag="mb")
            ocol = pp.tile([128, 1], bf16, tag="ocol")
            ident = pp.tile([128, 128], bf16, tag="ident")
            qt = pp.tile([128, S], bf16, tag="qt")
            kt = pp.tile([128, S], bf16, tag="kt")
            vt = pp.tile([128, S], bf16, tag="vt")
            vn = pp.tile([128, S], bf16, tag="vn")
            wrm_i = pp.tile([1, 32], f32, tag="wrm_i")
            wrm_o = pp.tile([1, 32], f32, tag="wrm_o")

            # exp table preload (overlaps the input DMA)
            nc.vector.memset(wrm_i[:], 0.0)
            nc.scalar.activation(wrm_o[:], wrm_i[:], Exp)

            nc.vector.memset(ocol[:], 1.0)
            make_identity(nc, ident[:])

            # DMAs, split across the two HWDGE queues.
            xt3 = xt_d.ap().rearrange("(c p) m -> p c m", p=128)
            nc.sync.dma_start(wk[:], wk_d.ap())
            nc.sync.dma_start(wq[:], wq_d.ap())
            for c in (0, 2, 4, 6):
                nc.sync.dma_start(xts[c][:], xt3[:, c, :])
            nc.scalar.dma_start(wv[:], wv_d.ap())
            for c in (1, 3, 5, 7):
                nc.scalar.dma_start(xts[c][:], xt3[:, c, :])
            nc.scalar.dma_start(bcol[:], bcol_d.ap())
            nc.scalar.dma_start(mb[:], mb_d.ap())

            def cast_bias(dst_sl, src, col):
                nc.vector.tensor_scalar_add(dst_sl, src, bcol[:, col:col + 1])

            egs = {}      # (pair, j) -> E tile [128, 1024] bf16, pre-masked
            rlast = {}    # pair -> latest running-sum tile [128, 1024] bf16
            pds = {}      # t -> denominator PSUM tile [1, 512]
            rdbs = {}     # t -> broadcast reciprocal [128, 512] f32
            pots = {}     # t -> AV accumulator PSUM tile

            # ---------- pass 1: KT (all blocks) + QT blocks 0,1 ----------
            with tc.tile_pool(name="psA1", bufs=1, space="PSUM") as psA1:
                psK = [psA1.tile([128, QBLK], f32, tag=f"pk{t}", name=f"pk{t}")
                       for t in range(NQB)]
                psQ0 = psA1.tile([128, QBLK], f32, tag="pq0")
                psQ1 = psA1.tile([128, QBLK], f32, tag="pq1")
                for c in range(NCHUNK):
                    wks = wk[:, c * 128:(c + 1) * 128]
                    wqs = wq[:, c * 128:(c + 1) * 128]
                    st, sp = (c == 0), (c == NCHUNK - 1)
                    nc.tensor.matmul(psK[0][:], wks, xts[c][:, 0:512],
                                     start=st, stop=sp)
                    nc.tensor.matmul(psK[1][:], wks, xts[c][:, 512:1024],
                                     start=st, stop=sp)
                    nc.tensor.matmul(psQ0[:], wqs, xts[c][:, 0:512],
                                     start=st, stop=sp)
                    nc.tensor.matmul(psK[2][:], wks, xts[c][:, 1024:1536],
                                     start=st, stop=sp)
                    nc.tensor.matmul(psK[3][:], wks, xts[c][:, 1536:2048],
                                     start=st, stop=sp)
                    nc.tensor.matmul(psQ1[:], wqs, xts[c][:, 512:1024],
                                     start=st, stop=sp)
                for t in range(NQB):
                    cast_bias(kt[:, t * QBLK:(t + 1) * QBLK], psK[t][:], 1)
                cast_bias(qt[:, 0:QBLK], psQ0[:], 0)
                cast_bias(qt[:, QBLK:2 * QBLK], psQ1[:], 0)

            # ---------- phase C pools (ps_st lives through the R region) ----
            with tc.tile_pool(name="ps_st", bufs=2, space="PSUM") as ps_st:

                def emit_score_pair(pair, j):
                    """pair 0 -> q blocks 0,1; pair 1 -> q blocks 2,3."""
                    pst = ps_st.tile([128, 2 * QBLK], f32, tag="st",
                                     name=f"pst{pair}_{j}")
                    ktj = kt[:, j * 128:(j + 1) * 128]
                    for half in (0, 1):
                        t = 2 * pair + half
                        nc.tensor.matmul(
                            pst[:, half * QBLK:(half + 1) * QBLK],
                            ktj, qt[:, t * QBLK:(t + 1) * QBLK],
                            start=True, stop=True,
                        )
                    eg = ep.tile([128, 2 * QBLK], bf16, tag="e",
                                 name=f"eg{pair}_{j}")
                    nc.scalar.activation(eg[:], pst[:], Exp,
                                         bias=mb[:, j:j + 1])
                    egs[(pair, j)] = eg

                def emit_tree(pair, j):
                    """running masked-E sum for a block pair (plain adds)."""
                    eg = egs[(pair, j)]
                    if j == 0:
                        rlast[pair] = eg
                        return
                    r = tp.tile([128, 2 * QBLK], bf16, tag="r",
                                name=f"r{pair}_{j}")
                    nc.vector.tensor_add(r[:], rlast[pair][:], eg[:])
                    rlast[pair] = r

                def emit_av(t, j):
                    if j == 0:
                        pots[t] = ps_o.tile([128, QBLK], f32, tag="o",
                                            name=f"pot{t}")
                    eg = egs[(t // 2, j)]
                    half = t % 2
                    nc.tensor.matmul(
                        pots[t][:],
                        vn[:, j * 128:(j + 1) * 128],
                        eg[:, half * QBLK:(half + 1) * QBLK],
                        start=(j == 0), stop=(j == NKEY - 1),
                    )

                def emit_pd(t):
                    """denominator matmul + reciprocal + broadcast (early)."""
                    half = t % 2
                    pd = ps_m.tile([1, QBLK], f32, tag="d", name=f"pd{t}")
                    nc.tensor.matmul(
                        pd[:], ocol[:],
                        rlast[t // 2][:, half * QBLK:(half + 1) * QBLK],
                        start=True, stop=True)
                    rdc = rp.tile([1, QBLK], f32, tag="rdc", name=f"rdc{t}")
                    nc.vector.reciprocal_approx_fast(rdc[:], pd[:])
                    rdb = rp.tile([128, QBLK], f32, tag="rdb", name=f"rdb{t}")
                    nc.gpsimd.partition_broadcast(rdb[:], rdc[:])
                    rdbs[t] = rdb

                def emit_finish(t):
                    osb = op.tile([128, QBLK], f32, tag="osb", name=f"osb{t}")
                    nc.vector.tensor_mul(osb[:], pots[t][:], rdbs[t][:])
                    nc.sync.dma_start(
                        out_d.ap()[:, t * QBLK:(t + 1) * QBLK], osb[:])

                # ---- R region: rest of projections + V transposes,
                # interleaved with score pairs for blocks 0,1 ----
                with (
                    tc.tile_pool(name="psA2", bufs=2, space="PSUM") as psA2,
                    tc.tile_pool(name="ps_tr", bufs=2, space="PSUM") as ps_tr,
                ):
                    r_ops = []  # PE-op thunks, 4 consumed per score pair

                    def proj8(w, t, dst, col, nm):
                        pr = psA2.tile([128, QBLK], f32, tag="pr",
                                       name=f"pr_{nm}")
                        for c in range(NCHUNK):
                            r_ops.append(lambda c=c, pr=pr: nc.tensor.matmul(
                                pr[:],
                                w[:, c * 128:(c + 1) * 128],
                                xts[c][:, t * QBLK:(t + 1) * QBLK],
                                start=(c == 0), stop=(c == NCHUNK - 1)))
                        r_ops.append(lambda pr=pr: cast_bias(
                            dst[:, t * QBLK:(t + 1) * QBLK], pr[:], col))

                    def tr1(k):
                        ptr = ps_tr.tile([128, 128], bf16, tag="tr",
                                         name=f"tr{k}")
                        nc.tensor.transpose(
                            ptr[:], vt[:, k * 128:(k + 1) * 128], ident[:])
                        nc.vector.tensor_copy(
                            vn[:, k * 128:(k + 1) * 128], ptr[:])

                    proj8(wq, 2, qt, 0, "q2")
                    proj8(wq, 3, qt, 0, "q3")
                    for t in range(NQB):
                        proj8(wv, t, vt, 2, f"v{t}")
                        for k in range(4 * t, 4 * t + 4):
                            r_ops.append(lambda k=k: tr1(k))

                    ri = 0
                    for j in range(NKEY):
                        emit_score_pair(0, j)
                        emit_tree(0, j)
                        for _ in range(4):
                            if ri < len(r_ops):
                                r_ops[ri]()
                                ri += 1
                    while ri < len(r_ops):
                        r_ops[ri]()
                        ri += 1

                # ---- steady phase C ----
                # PSUM budget: ps_st 8KB + 3 concurrent pots 6KB + pd 2KB
                # = 16KB exactly, so AV for block 3 trails the loop.
                with (
                    tc.tile_pool(name="ps_o", bufs=3, space="PSUM") as ps_o,
                    tc.tile_pool(name="ps_m", bufs=1, space="PSUM") as ps_m,
                ):
                    for j in range(NKEY):
                        emit_score_pair(1, j)
                        emit_av(0, j)
                        emit_av(1, j)
                        if j >= 2:
                            emit_av(2, j - 2)
                        if j == 2:
                            emit_pd(0)
                        if j == 4:
                            emit_pd(1)
                        emit_tree(1, j)
                    emit_av(2, NKEY - 2)
                    emit_av(2, NKEY - 1)
                    emit_finish(0)
                    emit_finish(1)
                    emit_pd(2)
                    for j in range(0, NKEY // 2):
                        emit_av(3, j)
                    emit_pd(3)
                    for j in range(NKEY // 2, NKEY):
                        emit_av(3, j)
                    emit_finish(2)
                    emit_finish(3)

    nc.compile()
    return nc


_NC = None


def _get_nc():
    global _NC
    if _NC is None:
        _NC = _build()
    return _NC


def _prep_in_maps(input_tensor, attention_mask, Wq, bq, Wk, bk, Wv, bv):
    scale = np.float32(1.0 / np.sqrt(np.float32(S)))

    def pack_w(w, sc=None):
        w = np.asarray(w, np.float32)
        if sc is not None:
            w = w * sc
        # [1024, 128] -> [128, 8*128]: row c*128+p, col e -> [p, c*128+e]
        return np.ascontiguousarray(
            w.reshape(NCHUNK, 128, DOUT).transpose(1, 0, 2).reshape(128, DIN)
        ).astype(BF16)

    wq_h = pack_w(Wq, scale)
    wk_h = pack_w(Wk)
    wv_h = pack_w(Wv)
    bcol_h = np.zeros((128, 4), np.float32)
    bcol_h[:, 0] = np.asarray(bq, np.float32) * scale
    bcol_h[:, 1] = np.asarray(bk, np.float32)
    bcol_h[:, 2] = np.asarray(bv, np.float32)

    x = np.asarray(input_tensor, np.float32)
    m = np.asarray(attention_mask)
    in_maps = []
    for b in range(B):
        xt_h = np.ascontiguousarray(x[b].T).astype(BF16)            # [DIN, S]
        # exp bias per (key % 128, key chunk): 0 keep, -50 mask
        mb_h = np.ascontiguousarray(
            (m[b].astype(np.float32).reshape(NKEY, 128).T - 1.0) * 50.0)
        in_maps.append({
            "xt": xt_h, "wq": wq_h, "wk": wk_h, "wv": wv_h,
            "bcol": bcol_h, "mb": mb_h,
        })
    return in_maps


def run(in_maps, trace=False, **kwargs):
    from concourse.bass_utils import run_bass_kernel_spmd

    nc = _get_nc()
    return run_bass_kernel_spmd(
        nc, in_maps, core_ids=list(range(B)), trace=trace, **kwargs
    )


def kernel(input_tensor, attention_mask, Wq, bq, Wk, bk, Wv, bv):
    in_maps = _prep_in_maps(
        input_tensor, attention_mask, Wq, bq, Wk, bk, Wv, bv)
    res = run(in_maps, trace=False)
    out = np.stack([res.results[b]["out"].T for b in range(B)])
    return np.ascontiguousarray(out.astype(np.float32))


# revision 6
# speedup vs baseline: 1.4926x; 1.0651x over previous
"""AttentionHead kernel for Trainium2, 8 NeuronCores, data-parallel over batch.

Problem (fixed shapes):
    input_tensor [8, 2048, 1024] f32, attention_mask [8, 2048] int64 (0/1),
    Wq/Wk/Wv [1024, 128] f32, bq/bk/bv [128] f32.
    out = softmax(mask(Q @ K^T / sqrt(2048))) @ V    -> [8, 2048, 128] f32

Sharding: one batch element per core (B == n_cores == 8). No collectives.

Per-core device kernel (bf16 inputs, f32 accumulation), v3:
  - Host pre-transposes X -> XT [1024, 2048] bf16, prepacks W into the SBUF
    layout [128, 8*128] (contiguous 2KB partition lines -> fast DMA), folds
    1/sqrt(S) into Wq/bq.
  - DMA issue is split across the two HWDGE queues (sync + scalar) so issue
    overhead (~0.6us per DMA instruction) overlaps; a dummy exp at t=0
    preloads the ACT exp table during the X DMA.
  - Pass 1 (DMA-paced, per chunk): KT all 4 blocks + QT blocks 0,1.
    Bias is folded into the PSUM->SBUF cast (per-partition tensor_scalar).
  - Score/exp tiles cover ONE key chunk x TWO query blocks [128, 1024]:
    within a tile the mask depends only on the partition (the key), so the
    attention mask folds into the exp's per-partition bias (0 or -50) and
    E comes out of ScalarE already masked. 32 activations total; no
    max-subtraction (|scores| <= ~1.5 by construction).
  - Softmax denominator: plain running TENSOR_TENSOR adds over the 16
    pre-masked E tiles per block pair, one [K=128,M=1,N=512] matmul with a
    ones column, reciprocal_approx_fast (DVE), gpsimd partition_broadcast,
    final DVE multiply. The chain is emitted early so it overlaps AV.
  - Emission interleaves: scores(blocks 0,1) into the remaining projections;
    scores(blocks 2,3) + AV(0..3) in one loop, AV of blocks 2,3 lagging two
    key chunks so every engine stays busy. PE never idles > ~1us, so the
    HAM clock gate stays open.
  - Output written as OT [128, 2048]; host transposes to [2048, 128].
"""

import sys

for _p in ("/opt/trn_rl_repo", "/root/.axon_site/_ro/trn_rl_repo"):
    if _p not in sys.path:
        sys.path.append(_p)

import numpy as np
import ml_dtypes

B, S, DIN, DOUT = 8, 2048, 1024, 128
NCHUNK = DIN // 128          # 8 contraction chunks
NKEY = S // 128              # 16 key chunks
QBLK = 512                   # query block (free dim of S^T / OT matmuls)
NQB = S // QBLK              # 4 query blocks

BF16 = ml_dtypes.bfloat16


def _build():
    import concourse.bass as bass
    import concourse.tile as tile
    from concourse import bacc, mybir
    from concourse.masks import make_identity

    f32 = mybir.dt.float32
    bf16 = mybir.dt.bfloat16
    Exp = mybir.ActivationFunctionType.Exp

    nc = bacc.Bacc("TRN2", target_bir_lowering=False, debug=False, num_devices=B)

    xt_d = nc.dram_tensor("xt", [DIN, S], bf16, kind="ExternalInput")
    wq_d = nc.dram_tensor("wq", [128, DIN], bf16, kind="ExternalInput")
    wk_d = nc.dram_tensor("wk", [128, DIN], bf16, kind="ExternalInput")
    wv_d = nc.dram_tensor("wv", [128, DIN], bf16, kind="ExternalInput")
    bcol_d = nc.dram_tensor("bcol", [128, 4], f32, kind="ExternalInput")
    mb_d = nc.dram_tensor("mb", [128, NKEY], f32, kind="ExternalInput")
    out_d = nc.dram_tensor("out", [DOUT, S], f32, kind="ExternalOutput")

    with tile.TileContext(nc) as tc:
        with (
            tc.tile_pool(name="persist", bufs=1) as pp,
            tc.tile_pool(name="epool", bufs=20) as ep,
            tc.tile_pool(name="tree", bufs=6) as tp,
            tc.tile_pool(name="normp", bufs=2) as rp,
            tc.tile_pool(name="outp", bufs=2) as op,
        ):
            xts = [pp.tile([128, S], bf16, tag=f"xt{c}", name=f"xt{c}")
                   for c in range(NCHUNK)]
            wq = pp.tile([128, DIN], bf16, tag="wq")
            wk = pp.tile([128, DIN], bf16, tag="wk")
            wv = pp.tile([128, DIN], bf16, tag="wv")
            bcol = pp.tile([128, 4], f32, tag="bcol")
            mb = pp.tile([128, NKEY], f32, tag="mb")
            ocol = pp.tile([128, 1], bf16, tag="ocol")
            ident = pp.tile([128, 128], bf16, tag="ident")
            qt = pp.tile([128, S], bf16, tag="qt")
            kt = pp.tile([128, S], bf16, tag="kt")
            vt = pp.tile([128, S], bf16, tag="vt")
            vn = pp.tile([128, S], bf16, tag="vn")
            wrm_i = pp.tile([1, 32], f32, tag="wrm_i")
            wrm_o = pp.tile([1, 32], f32, tag="wrm_o")

            # exp table preload (overlaps the input DMA)
            nc.vector.memset(wrm_i[:], 0.0)
            nc.scalar.activation(wrm_o[:], wrm_i[:], Exp)

            nc.vector.memset(ocol[:], 1.0)
            make_identity(nc, ident[:])

            # DMAs. All X chunks go on the sync queue in consumption order:
            # serial issue (~0.7us each) keeps only ~2 transfers in flight,
            # so chunks complete near-sequentially (packet round-robin would
            # otherwise delay chunk 0 to the end). Small tensors ride the
            # scalar queue.
            xt3 = xt_d.ap().rearrange("(c p) m -> p c m", p=128)
            nc.sync.dma_start(wk[:], wk_d.ap())
            nc.sync.dma_start(wq[:], wq_d.ap())
            for c in range(NCHUNK):
                nc.sync.dma_start(xts[c][:], xt3[:, c, :])
            nc.scalar.dma_start(mb[:], mb_d.ap())
            nc.scalar.dma_start(bcol[:], bcol_d.ap())
            nc.scalar.dma_start(wv[:], wv_d.ap())

            def cast_bias(dst_sl, src, col):
                nc.vector.tensor_scalar_add(dst_sl, src, bcol[:, col:col + 1])

            egs = {}      # (pair, j) -> E tile [128, 1024] bf16, pre-masked
            rlast = {}    # pair -> latest running-sum tile [128, 1024] bf16
            pds = {}      # t -> denominator PSUM tile [1, 512]
            rdbs = {}     # t -> broadcast reciprocal [128, 512] f32
            pots = {}     # t -> AV accumulator PSUM tile

            # ---------- pass 1: KT (all blocks) + QT blocks 0,1 ----------
            with tc.tile_pool(name="psA1", bufs=1, space="PSUM") as psA1:
                psK = [psA1.tile([128, QBLK], f32, tag=f"pk{t}", name=f"pk{t}")
                       for t in range(NQB)]
                psQ0 = psA1.tile([128, QBLK], f32, tag="pq0")
                psQ1 = psA1.tile([128, QBLK], f32, tag="pq1")
                for c in range(NCHUNK):
                    wks = wk[:, c * 128:(c + 1) * 128]
                    wqs = wq[:, c * 128:(c + 1) * 128]
                    st, sp = (c == 0), (c == NCHUNK - 1)
                    nc.tensor.matmul(psK[0][:], wks, xts[c][:, 0:512],
                                     start=st, stop=sp)
                    nc.tensor.matmul(psK[1][:], wks, xts[c][:, 512:1024],
                                     start=st, stop=sp)
                    nc.tensor.matmul(psQ0[:], wqs, xts[c][:, 0:512],
                                     start=st, stop=sp)
                    nc.tensor.matmul(psK[2][:], wks, xts[c][:, 1024:1536],
                                     start=st, stop=sp)
                    nc.tensor.matmul(psK[3][:], wks, xts[c][:, 1536:2048],
                                     start=st, stop=sp)
                    nc.tensor.matmul(psQ1[:], wqs, xts[c][:, 512:1024],
                                     start=st, stop=sp)
                for t in range(NQB):
                    cast_bias(kt[:, t * QBLK:(t + 1) * QBLK], psK[t][:], 1)
                cast_bias(qt[:, 0:QBLK], psQ0[:], 0)
                cast_bias(qt[:, QBLK:2 * QBLK], psQ1[:], 0)

            # ---------- phase C pools (ps_st lives through the R region) ----
            with tc.tile_pool(name="ps_st", bufs=2, space="PSUM") as ps_st:

                def emit_score_pair(pair, j):
                    """pair 0 -> q blocks 0,1; pair 1 -> q blocks 2,3."""
                    pst = ps_st.tile([128, 2 * QBLK], f32, tag="st",
                                     name=f"pst{pair}_{j}")
                    ktj = kt[:, j * 128:(j + 1) * 128]
                    for half in (0, 1):
                        t = 2 * pair + half
                        nc.tensor.matmul(
                            pst[:, half * QBLK:(half + 1) * QBLK],
                            ktj, qt[:, t * QBLK:(t + 1) * QBLK],
                            start=True, stop=True,
                        )
                    eg = ep.tile([128, 2 * QBLK], bf16, tag="e",
                                 name=f"eg{pair}_{j}")
                    nc.scalar.activation(eg[:], pst[:], Exp,
                                         bias=mb[:, j:j + 1])
                    egs[(pair, j)] = eg

                def emit_tree(pair, j):
                    """running masked-E sum for a block pair (plain adds)."""
                    eg = egs[(pair, j)]
                    if j == 0:
                        rlast[pair] = eg
                        return
                    r = tp.tile([128, 2 * QBLK], bf16, tag="r",
                                name=f"r{pair}_{j}")
                    nc.vector.tensor_add(r[:], rlast[pair][:], eg[:])
                    rlast[pair] = r

                def emit_av(t, j):
                    if j == 0:
                        pots[t] = ps_o.tile([128, QBLK], f32, tag="o",
                                            name=f"pot{t}")
                    eg = egs[(t // 2, j)]
                    half = t % 2
                    nc.tensor.matmul(
                        pots[t][:],
                        vn[:, j * 128:(j + 1) * 128],
                        eg[:, half * QBLK:(half + 1) * QBLK],
                        start=(j == 0), stop=(j == NKEY - 1),
                    )

                def emit_pd(t):
                    """denominator matmul + reciprocal + broadcast (early)."""
                    half = t % 2
                    pd = ps_m.tile([1, QBLK], f32, tag="d", name=f"pd{t}")
                    nc.tensor.matmul(
                        pd[:], ocol[:],
                        rlast[t // 2][:, half * QBLK:(half + 1) * QBLK],
                        start=True, stop=True)
                    rdc = rp.tile([1, QBLK], f32, tag="rdc", name=f"rdc{t}")
                    nc.vector.reciprocal_approx_fast(rdc[:], pd[:])
                    rdb = rp.tile([128, QBLK], f32, tag="rdb", name=f"rdb{t}")
                    nc.gpsimd.partition_broadcast(rdb[:], rdc[:])
                    rdbs[t] = rdb

                def emit_finish(t):
                    osb = op.tile([128, QBLK], f32, tag="osb", name=f"osb{t}")
                    nc.vector.tensor_mul(osb[:], pots[t][:], rdbs[t][:])
                    nc.sync.dma_start(
                        out_d.ap()[:, t * QBLK:(t + 1) * QBLK], osb[:])

                # ---- R region: rest of projections + V transposes,
                # interleaved with score pairs for blocks 0,1 ----
                with (
                    tc.tile_pool(name="psA2", bufs=2, space="PSUM") as psA2,
                    tc.tile_pool(name="ps_tr", bufs=2, space="PSUM") as ps_tr,
                ):
                    r_ops = []  # PE-op thunks, 4 consumed per score pair

                    def proj8(w, t, dst, col, nm):
                        pr = psA2.tile([128, QBLK], f32, tag="pr",
                                       name=f"pr_{nm}")
                        for c in range(NCHUNK):
                            r_ops.append(lambda c=c, pr=pr: nc.tensor.matmul(
                                pr[:],
                                w[:, c * 128:(c + 1) * 128],
                                xts[c][:, t * QBLK:(t + 1) * QBLK],
                                start=(c == 0), stop=(c == NCHUNK - 1)))
                        r_ops.append(lambda pr=pr: cast_bias(
                            dst[:, t * QBLK:(t + 1) * QBLK], pr[:], col))

                    def tr1(k):
                        ptr = ps_tr.tile([128, 128], bf16, tag="tr",
                                         name=f"tr{k}")
                        nc.tensor.transpose(
                            ptr[:], vt[:, k * 128:(k + 1) * 128], ident[:])
                        nc.vector.tensor_copy(
                            vn[:, k * 128:(k + 1) * 128], ptr[:])

                    proj8(wq, 2, qt, 0, "q2")
                    proj8(wq, 3, qt, 0, "q3")
                    for t in range(NQB):
                        proj8(wv, t, vt, 2, f"v{t}")
                        for k in range(4 * t, 4 * t + 4):
                            r_ops.append(lambda k=k: tr1(k))

                    ri = 0
                    for j in range(NKEY):
                        emit_score_pair(0, j)
                        emit_tree(0, j)
                        for _ in range(4):
                            if ri < len(r_ops):
                                r_ops[ri]()
                                ri += 1
                    while ri < len(r_ops):
                        r_ops[ri]()
                        ri += 1

                # ---- steady phase C ----
                # PSUM budget: ps_st 8KB + 3 concurrent pots 6KB + pd 2KB
                # = 16KB exactly, so AV for block 3 trails the loop.
                with (
                    tc.tile_pool(name="ps_o", bufs=3, space="PSUM") as ps_o,
                    tc.tile_pool(name="ps_m", bufs=1, space="PSUM") as ps_m,
                ):
                    for j in range(NKEY):
                        emit_score_pair(1, j)
                        emit_av(0, j)
                        emit_av(1, j)
                        if j >= 2:
                            emit_av(2, j - 2)
                        if j == 2:
                            emit_pd(0)
                        if j == 4:
                            emit_pd(1)
                        emit_tree(1, j)
                    emit_av(2, NKEY - 2)
                    emit_av(2, NKEY - 1)
                    emit_finish(0)
                    emit_finish(1)
                    emit_pd(2)
                    for j in range(0, NKEY // 2):
                        emit_av(3, j)
                    emit_pd(3)
                    for j in range(NKEY // 2, NKEY):
                        emit_av(3, j)
                    emit_finish(2)
                    emit_finish(3)

    nc.compile()
    return nc


_NC = None


def _get_nc():
    global _NC
    if _NC is None:
        _NC = _build()
    return _NC


def _prep_in_maps(input_tensor, attention_mask, Wq, bq, Wk, bk, Wv, bv):
    scale = np.float32(1.0 / np.sqrt(np.float32(S)))

    def pack_w(w, sc=None):
        w = np.asarray(w, np.float32)
        if sc is not None:
            w = w * sc
        # [1024, 128] -> [128, 8*128]: row c*128+p, col e -> [p, c*128+e]
        return np.ascontiguousarray(
            w.reshape(NCHUNK, 128, DOUT).transpose(1, 0, 2).reshape(128, DIN)
        ).astype(BF16)

    wq_h = pack_w(Wq, scale)
    wk_h = pack_w(Wk)
    wv_h = pack_w(Wv)
    bcol_h = np.zeros((128, 4), np.float32)
    bcol_h[:, 0] = np.asarray(bq, np.float32) * scale
    bcol_h[:, 1] = np.asarray(bk, np.float32)
    bcol_h[:, 2] = np.asarray(bv, np.float32)

    x = np.asarray(input_tensor, np.float32)
    m = np.asarray(attention_mask)
    in_maps = []
    for b in range(B):
        xt_h = np.ascontiguousarray(x[b].T).astype(BF16)            # [DIN, S]
        # exp bias per (key % 128, key chunk): 0 keep, -50 mask
        mb_h = np.ascontiguousarray(
            (m[b].astype(np.float32).reshape(NKEY, 128).T - 1.0) * 50.0)
        in_maps.append({
            "xt": xt_h, "wq": wq_h, "wk": wk_h, "wv": wv_h,
            "bcol": bcol_h, "mb": mb_h,
        })
    return in_maps


def run(in_maps, trace=False, **kwargs):
    from concourse.bass_utils import run_bass_kernel_spmd

    nc = _get_nc()
    return run_bass_kernel_spmd(
        nc, in_maps, core_ids=list(range(B)), trace=trace, **kwargs
    )


def kernel(input_tensor, attention_mask, Wq, bq, Wk, bk, Wv, bv):
    in_maps = _prep_in_maps(
        input_tensor, attention_mask, Wq, bq, Wk, bk, Wv, bv)
    res = run(in_maps, trace=False)
    out = np.stack([res.results[b]["out"].T for b in range(B)])
    return np.ascontiguousarray(out.astype(np.float32))
